# revision 27
# baseline (speedup 1.0000x reference)
"""Causal self-attention (RoPE, 16 heads, d=64, B=4, T=2048, C=1024) on 8 TRN2 cores.

Sharding: core g = (batch b = g//2, head-group hg = g%2 covering 8 heads).
Data-parallel over B, tensor-parallel over heads.  Each core computes the
partial out-projection (its 8 heads' contribution, no bias); the host sums
the two head-group partials per batch and adds b_out.

v1 changes over the 419.5us baseline:
  - input loads split across gpsimd/vector/scalar DMA queues in need-order
    (was: one serial gpsimd queue -> ~40us dead start)
  - cos/sin tables stored compact [T, 64] and broadcast across heads in the
    rope multiplies (was: 8x duplicated, 4MB of DMA)
  - diagonal-block masking moved post-exp: S computed unmasked in one merged
    matmul pair per diagonal j, then the upper-triangle of exp(S) is zeroed
    by a DVE multiply with a 0/1 tri mask (was: 2 seed matmuls + 2 split S
    matmuls per diagonal block, with iden/negtri LDWEIGHTS churn)
  - softmax normalize fused: reciprocal_approx_fast (5x faster than the 6
    cyc/elem DVE reciprocal) on the denominator + scalar_tensor_tensor
    reading the attention half straight from PSUM (was: 2 evac copies +
    3.3us reciprocal + gpsimd multiply per half)
  - v stored per-h2-plane ([v|ones] for even heads, [ones|v] for odd) so the
    AV output lands attn on the same partitions the out-projection reads,
    letting the fused normalize write aT directly
  - out-projection PSUM evacuation on gpsimd; output stored bf16
"""

import os
from contextlib import ExitStack

import numpy as np
import ml_dtypes

B, T, C = 4, 2048, 1024
H, D = 16, 64
HG = 8            # heads per core
NCORES = 8
TB = T // 128     # 16 t/s-blocks of 128
CBN = C // 128    # 8 contraction chunks
NP = HG // 2      # 4 head pairs
NI = T // 512     # 4 t-windows of 512
ROPE_BASE = 10000.0

_PROG = None
_LAST_RESULTS = None


def _build_program():
    import concourse.bass as bass
    import concourse.tile as tile
    from concourse import bacc, mybir

    f32 = mybir.dt.float32
    bf = mybir.dt.bfloat16
    EXP = mybir.ActivationFunctionType.Exp
    MUL = mybir.AluOpType.mult

    nc = bacc.Bacc("TRN2", target_bir_lowering=False, debug=False)

    xT = nc.dram_tensor("xT", [C, T], bf, kind="ExternalInput").ap()
    wqkv = nc.dram_tensor("wqkv", [C, 3 * HG * D], bf, kind="ExternalInput").ap()
    wout = nc.dram_tensor("wout", [HG * D, C], bf, kind="ExternalInput").ap()
    cosc = nc.dram_tensor("cosc", [T, D], bf, kind="ExternalInput").ap()
    sinc = nc.dram_tensor("sinc", [T, D], bf, kind="ExternalInput").ap()
    padb = nc.dram_tensor("padb", [128, TB], f32, kind="ExternalInput").ap()
    tri = nc.dram_tensor("tri", [128, 128], bf, kind="ExternalInput").ap()
    outp = nc.dram_tensor("outp", [T, C], bf, kind="ExternalOutput").ap()

    with tile.TileContext(nc) as tc, ExitStack() as ctx:
        singles = ctx.enter_context(tc.tile_pool(name="singles", bufs=1))

        # ---- global SBUF tensors.  Loads are split across three DMA queues
        # (gpsimd: xT, vector: wqkv, scalar: the rest) and issued in the
        # order compute consumes them, so the first qkv matmul can start
        # ~2us in instead of waiting out one serial queue.  The Sync queue
        # carries ONLY DMA transposes so the xbar never flips modes. ----
        # Load-queue plan: single merged tiles so each load is ONE DMA
        # trigger (triggers cost ~0.7us gpsimd/sync, ~1.3us scalar, and
        # serialize per queue).  Segment 0's inputs all land by ~6us:
        #   sync:   w halves 0:4      (before the first transpose is issued,
        #                              so the xbar never flips modes)
        #   scalar: w halves 4:8, cos, sin, wout
        #   gpsimd: xt q0, padb, tri, xt q1..q3  (t-quarters in consumption
        #                              order: segment I only needs quarter I)
        xt_sb = []
        w_sb = []
        for cb in range(CBN):
            t_ = singles.tile([128, T], bf, name=f"xt{cb}", tag=f"xt{cb}")
            xt_sb.append(t_)
            t_ = singles.tile([128, 3 * HG * D], bf, name=f"w{cb}", tag=f"w{cb}")
            eng = nc.sync if cb < 6 else nc.scalar
            eng.dma_start(out=t_, in_=wqkv[cb * 128:(cb + 1) * 128, :])
            w_sb.append(t_)
        cos_sb = singles.tile([128, TB, D], bf, name="cos_sb", tag="cos_sb")
        nc.scalar.dma_start(
            out=cos_sb, in_=cosc.rearrange("(tb p) d -> p tb d", p=128))
        sin_sb = singles.tile([128, TB, D], bf, name="sin_sb", tag="sin_sb")
        nc.scalar.dma_start(
            out=sin_sb, in_=sinc.rearrange("(tb p) d -> p tb d", p=128))
        for q in range(NI):
            for cb in range(CBN):
                nc.gpsimd.dma_start(
                    out=xt_sb[cb][:, q * 512:(q + 1) * 512],
                    in_=xT[cb * 128:(cb + 1) * 128, q * 512:(q + 1) * 512])
            if q == 0:
                padb_sb = singles.tile([128, TB], f32, name="padb_sb", tag="padb_sb")
                nc.gpsimd.dma_start(out=padb_sb, in_=padb)
                tri_sb = singles.tile([128, 128], bf, name="tri_sb", tag="tri_sb")
                nc.gpsimd.dma_start(out=tri_sb, in_=tri)
        wo_sb = []
        for c in range(4):
            t_ = singles.tile([128, C], bf, name=f"wo{c}", tag=f"wo{c}")
            nc.scalar.dma_start(out=t_, in_=wout[c * 128:(c + 1) * 128, :])
            wo_sb.append(t_)

        # q^T/k^T: [within-pair col (head-lo d / head-hi d), s-block, pair, t]
        qT_all = singles.tile([128, TB, NP, 128], bf, name="qT_all", tag="qT_all")
        kT_all = singles.tile([128, TB, NP, 128], bf, name="kT_all", tag="kT_all")
        # v packed per h2-plane: plane 0 (even heads) [v | ones], plane 1
        # (odd heads) [ones | v].  The ones columns make the AV matmul emit
        # the softmax denominator on the opposite 64 partitions from attn,
        # and attn lands exactly on the partitions aT/out-proj read, so the
        # fused normalize (recip + scalar_tensor_tensor) writes aT directly.
        vones = singles.tile([128, TB, 2, NP, 128], bf, name="vones", tag="vones")
        nc.vector.memset(vones[:, :, 0, :, D:128], 1.0)
        nc.vector.memset(vones[:, :, 1, :, 0:D], 1.0)

        # qkv + attention interleaved per 512-wide t-window so the PE stream
        # stays dense (HAM stays at 2.4 GHz): attention for window I only
        # needs q/k/v blocks 0..4I+3, which segment I of the qkv loop topped
        # off.  One shared PSUM pool: qkv 2 + sAB 2x2 + psT 2 = 8 banks.
        with tc.tile_pool(name="psum", bufs=2, space="PSUM") as psum, \
             tc.tile_pool(name="rope", bufs=4) as rope_pool, \
             tc.tile_pool(name="qknat", bufs=3) as qk_pool, \
             tc.tile_pool(name="exps", bufs=3) as exp_pool, \
             tc.tile_pool(name="attnT", bufs=4) as aT_pool, \
             tc.tile_pool(name="recips", bufs=2) as rc_pool, \
             tc.tile_pool(name="outsb", bufs=4) as out_pool:
            # segment 0 runs with no attention to hide behind, so PE<->DVE
            # round trips on a 2-deep PSUM ring would halve its duty cycle.
            # The attention banks (sAB/avA/avB) are idle until window 0 --
            # borrow them for a ~6-deep ring during segment 0 only.
            SEG0_TAGS = (("qkv", 2), ("sAB", 2), ("avA", 1), ("avB", 1))

            def qkv_segment(I):
                """Generator emitting segment I's qkv matmuls in half-tensor
                chunks (yield points), so the caller can drip them into the
                attention loop as PE gap-filler while ACT grinds exps.

                q units first, then k, then v: window I+1's FIRST S matmul's
                moving operand spans all 4 new qT blocks, so the q
                transposes must clear the sync queue as early as possible
                (k blocks j>=4I+4 and v aren't touched until ~20us into the
                window)."""
                nchunk = 0
                for which, base in (("q", 0), ("k", 512), ("v", 1024)):
                    for tb in range(4 * I, 4 * I + 4):
                        if I == 0:
                            tg, nb = SEG0_TAGS[nchunk % len(SEG0_TAGS)]
                            nchunk += 1
                            ps = psum.tile([128, HG, D], f32,
                                           name=f"ps{which}", tag=tg, bufs=nb)
                        else:
                            ps = psum.tile([128, HG, D], f32, name=f"ps{which}", tag="qkv")
                        for cb in range(CBN):
                            nc.tensor.matmul(
                                ps, xt_sb[cb][:, tb * 128:(tb + 1) * 128],
                                w_sb[cb][:, base:base + 512],
                                start=(cb == 0), stop=(cb == CBN - 1))
                            if cb == 3:
                                yield
                        if which == "v":
                            # v columns were host-reordered [evens | odds].
                            # Early segments evacuate on ACT (idle then);
                            # late ones on DVE (ACT is exp-bound there).
                            if I <= 1:
                                nc.scalar.copy(
                                    out=vones[:, tb, 0, :, 0:D], in_=ps[:, 0:NP, :])
                                nc.scalar.copy(
                                    out=vones[:, tb, 1, :, D:128], in_=ps[:, NP:HG, :])
                            else:
                                nc.vector.tensor_copy(
                                    out=vones[:, tb, 0, :, 0:D], in_=ps[:, 0:NP, :])
                                nc.vector.tensor_copy(
                                    out=vones[:, tb, 1, :, D:128], in_=ps[:, NP:HG, :])
                            yield
                            continue
                        # rope: P_c = qkv*cos, P_s = qkv*sin (compact [t,64]
                        # tables broadcast across the 8 heads), then
                        # lo = P_c.lo - P_s.hi ; hi = P_s.lo + P_c.hi
                        cosb = cos_sb[:, tb].unsqueeze(1).broadcast_to([128, HG, D])
                        sinb = sin_sb[:, tb].unsqueeze(1).broadcast_to([128, HG, D])
                        pc = rope_pool.tile([128, HG, D], f32, name="pc", tag="rt")
                        psn = rope_pool.tile([128, HG, D], f32, name="psn", tag="rt")
                        nc.vector.tensor_mul(pc, ps, cosb)
                        nc.vector.tensor_mul(psn, ps, sinb)
                        ro = qk_pool.tile([128, HG, D], bf, name="ro", tag="ro")
                        nc.vector.tensor_sub(
                            ro[:, :, 0:32], pc[:, :, 0:32], psn[:, :, 32:64])
                        nc.vector.tensor_add(
                            ro[:, :, 32:64], psn[:, :, 0:32], pc[:, :, 32:64])
                        dst = qT_all if which == "q" else kT_all
                        nc.sync.dma_start_transpose(out=dst[:, tb, :, :], in_=ro)
                        yield

            pending_out = []
            for I in range(NI):
                nxt = qkv_segment(I + 1) if I + 1 < NI else None
                n_chunks = 24  # yield points per segment
                n_iters = 4 * (4 * I + 4)
                emitted = it = 0
                if I == 0:
                    for _ in qkv_segment(0):
                        pass
                    # window 0 is too short to absorb all of segment 1's
                    # matmul->rope->transpose pipeline; pre-drip half of it
                    # while the PE would otherwise idle waiting on window 0's
                    # first exps
                    while emitted < 12:
                        if next(nxt, "done") == "done":
                            break
                        emitted += 1

                def drip():
                    nonlocal emitted
                    got = 0
                    if nxt is None:
                        return got
                    # finish the segment by ~75% of the window (45% for the
                    # short window 0) so the next window's S matmuls never
                    # wait on the trailing rope->transpose pipeline
                    frac = 0.45 if I == 0 else 0.75
                    due = int(it * n_chunks / (frac * n_iters))
                    while emitted < due:
                        if next(nxt, "done") == "done":
                            break
                        emitted += 1
                        got += 1
                    return got

                # ---- attention window I ----
                aT_I = aT_pool.tile([128, NP, 512], bf, name="aT_I", tag="aT_I")
                for p in range(NP):
                    if nxt is None and p % 2 == 1:
                        # the last window has no qkv drip, so its 2 PSUM
                        # banks are free: odd pairs accumulate there, double-
                        # buffering the AV accumulators across pairs
                        psTA = psum.tile([128, 512], f32, name="psTA", tag="qkv", bufs=2)
                        psTB = psum.tile([128, 512], f32, name="psTB", tag="qkv", bufs=2)
                    else:
                        psTA = psum.tile([128, 512], f32, name="psTA", tag="avA", bufs=1)
                        psTB = psum.tile([128, 512], f32, name="psTB", tag="avB", bufs=1)

                    def emit_av(j, eAB):
                        off = max(j - 4 * I, 0) * 128
                        for h2, psT in ((0, psTA), (1, psTB)):
                            nc.tensor.matmul(
                                psT[:, off:512],
                                vones[:, j, h2, p, :],
                                eAB[:, h2, off:512],
                                start=(j == 0), stop=(j == 4 * I + 3))

                    prev = None
                    for j in range(4 * I + 4):
                        jl = j - 4 * I
                        off = max(jl, 0) * 128
                        sAB = psum.tile([128, 2, 512], f32, name="sAB", tag="sAB", bufs=2)
                        HALVES = ((0, slice(0, 64)), (1, slice(64, 128)))
                        # keep the row-tiled pair adjacent so the two heads
                        # overlap in the PE array; diagonal block computed
                        # unmasked (masked post-exp on DVE)
                        lo = max(jl, 0)
                        for h2, rows in HALVES:
                            nc.tensor.matmul(
                                sAB[:, h2, off:512],
                                kT_all[rows, j, p, :],
                                qT_all[rows, 4 * I + lo:4 * I + 4, p, :],
                                start=True, stop=True,
                                tile_position=(h2 * 64, 0))
                        eAB = exp_pool.tile([128, 2, 512], bf, name="eAB", tag="eAB")
                        bias = padb_sb[:, j:j + 1]
                        nc.scalar.activation(
                            out=eAB[:, :, off:512], in_=sAB[:, :, off:512],
                            func=EXP, bias=bias, scale=0.125)
                        if jl >= 0:
                            # zero exp(S) above the diagonal of the diagonal
                            # block (replaces the -1e30 seed matmuls); on
                            # gpsimd -- it is idle during windows and this
                            # keeps DVE off the S->AV critical path
                            trib = tri_sb.unsqueeze(1).broadcast_to([128, 2, 128])
                            nc.gpsimd.tensor_mul(
                                eAB[:, :, off:off + 128],
                                eAB[:, :, off:off + 128], trib)
                        if prev is not None:
                            emit_av(*prev)
                        prev = (j, eAB)
                        it += 1
                        # all out-projection units are deferred to the LAST
                        # window: exp work grows with causal depth, so window
                        # 3 is ACT-bound with ~24us of PE idle that the OP
                        # matmuls fill (earlier windows are PE-bound from the
                        # qkv drip).  Keep 2 units back for the tail so the
                        # PE has work while the final normalize chain drains.
                        if (nxt is None and it % 2 == 0
                                and len(pending_out) > 2):
                            pending_out.pop(0)()
                        drip()
                    emit_av(*prev)

                    # fused softmax normalize, straight out of PSUM into the
                    # out-projection's stationary layout: plane layout puts
                    # attn on the partitions aT reads (0:64 for h2=0, 64:128
                    # for h2=1) and the replicated denominator on the other
                    # 64.  Shift-copy the denominator onto attn's partitions,
                    # approx-reciprocal it (~51 ULP, plenty for softmax), then
                    # aT = psum_attn * recip in one scalar_tensor_tensor.
                    # (reciprocal_approx_fast silently breaks on partition-
                    # sliced APs, so gather both denominators into one tile
                    # and reciprocal all 128 partitions in one op)
                    cpd = rc_pool.tile([128, 512], f32, name="cpd", tag="rcd")
                    rc = rc_pool.tile([128, 512], f32, name="rc", tag="rc")
                    nc.vector.tensor_copy(out=cpd[0:64, :], in_=psTA[64:128, :])
                    nc.vector.tensor_copy(out=cpd[64:128, :], in_=psTB[0:64, :])
                    nc.vector.reciprocal_approx_fast(rc, cpd)
                    for h2, psT in ((0, psTA), (1, psTB)):
                        half = slice(h2 * 64, h2 * 64 + 64)
                        nc.vector.scalar_tensor_tensor(
                            out=aT_I[half, p, :], in0=psT[half, :], scalar=1.0,
                            in1=rc[half, :], op0=MUL, op1=MUL)

                # out-projection units are deferred into the next window's
                # loop as more PE gap-filler
                def make_out_unit(aT, i, il, n):
                    def emit():
                        pso = psum.tile([128, 512], f32, name="pso", tag="sAB", bufs=2)
                        for c in range(4):
                            nc.tensor.matmul(
                                pso,
                                aT[:, c, il * 128:(il + 1) * 128],
                                wo_sb[c][:, n * 512:(n + 1) * 512],
                                start=(c == 0), stop=(c == 3))
                        # evacuate on DVE: the units run in the ACT-bound
                        # last window, where ACT (exp) must not be loaded
                        # further and DVE has slack.  Stores alternate
                        # between the gpsimd and sync DMA queues (sync's
                        # transposes are all done by mid-window 2) so the
                        # deferred-store backlog doesn't stall the tail.
                        osb = out_pool.tile([128, 512], bf, name="osb", tag="osb")
                        nc.vector.tensor_copy(out=osb, in_=pso)
                        seng = nc.gpsimd if (i + n) % 2 == 0 else nc.sync
                        seng.dma_start(
                            out=outp[i * 128:(i + 1) * 128, n * 512:(n + 1) * 512],
                            in_=osb)
                    return emit
                for il in range(4):
                    for n in range(2):
                        pending_out.append(make_out_unit(aT_I, 4 * I + il, il, n))
                if nxt is not None:
                    for _ in nxt:
                        pass
            for f in pending_out:
                f()

    nc.compile()
    return nc


def _get_program():
    global _PROG
    if _PROG is None:
        _PROG = _build_program()
    return _PROG


def _rope_tables():
    bf16 = ml_dtypes.bfloat16
    inv = 1.0 / (ROPE_BASE ** (np.arange(0, D, 2, dtype=np.float64) / D))
    f = np.arange(T, dtype=np.float64)[:, None] * inv[None, :]  # [T, 32]
    c = np.cos(f)
    s = np.sin(f)
    # both 32-col halves carry the same table value
    cosc = np.concatenate([c, c], axis=1).astype(bf16)  # [T, 64]
    sinc = np.concatenate([s, s], axis=1).astype(bf16)
    return cosc, sinc


def kernel(x, attention_mask, W_qkv, W_out, b_out):
    global _LAST_RESULTS
    from concourse.bass_utils import run_bass_kernel_spmd

    nc = _get_program()
    bf16 = ml_dtypes.bfloat16
    x = np.asarray(x, dtype=np.float32)
    attention_mask = np.asarray(attention_mask)
    W_qkv = np.asarray(W_qkv, dtype=np.float32)
    W_out = np.asarray(W_out, dtype=np.float32)
    b_out = np.asarray(b_out, dtype=np.float32)

    cosc, sinc = _rope_tables()
    tri01 = np.where(np.arange(128)[:, None] > np.arange(128)[None, :], 0.0, 1.0)
    tri01 = tri01.astype(bf16)

    in_maps = []
    for g in range(NCORES):
        b, hg = g // 2, g % 2
        sl = slice(hg * 512, hg * 512 + 512)
        wq = W_qkv[:, 0 * C:][:, sl]
        wk = W_qkv[:, 1 * C:2 * C][:, sl]
        wv = W_qkv[:, 2 * C:3 * C][:, sl]
        # v columns reordered [even heads | odd heads] so the on-chip v
        # copies are two contiguous strided moves into the h2 planes
        wv = wv.reshape(C, HG, D)[:, [0, 2, 4, 6, 1, 3, 5, 7], :].reshape(C, 512)
        wqkv_g = np.ascontiguousarray(
            np.concatenate([wq, wk, wv], axis=1)).astype(bf16)
        xT_g = np.ascontiguousarray(x[b].T).astype(bf16)
        wout_g = np.ascontiguousarray(W_out[sl, :]).astype(bf16)
        padb_g = np.ascontiguousarray(
            np.where(attention_mask[b] != 0, 0.0, -1e30)
            .astype(np.float32).reshape(TB, 128).T)
        in_maps.append({
            "xT": xT_g, "wqkv": wqkv_g, "wout": wout_g,
            "cosc": cosc, "sinc": sinc, "padb": padb_g, "tri": tri01,
        })

    res = run_bass_kernel_spmd(nc, in_maps, list(range(NCORES)))
    _LAST_RESULTS = res
    out = np.empty((B, T, C), dtype=np.float32)
    for b in range(B):
        out[b] = (res.results[2 * b]["outp"].astype(np.float32)
                  + res.results[2 * b + 1]["outp"].astype(np.float32) + b_out)
    return out


# revision 29
# speedup vs baseline: 1.0043x; 1.0043x over previous
"""Causal self-attention (RoPE, 16 heads, d=64, B=4, T=2048, C=1024) on 8 TRN2 cores.

Sharding: core g = (batch b = g//2, head-group hg = g%2 covering 8 heads).
Data-parallel over B, tensor-parallel over heads.  Each core computes the
partial out-projection (its 8 heads' contribution, no bias); the host sums
the two head-group partials per batch and adds b_out.

v1 changes over the 419.5us baseline:
  - input loads split across gpsimd/vector/scalar DMA queues in need-order
    (was: one serial gpsimd queue -> ~40us dead start)
  - cos/sin tables stored compact [T, 64] and broadcast across heads in the
    rope multiplies (was: 8x duplicated, 4MB of DMA)
  - diagonal-block masking moved post-exp: S computed unmasked in one merged
    matmul pair per diagonal j, then the upper-triangle of exp(S) is zeroed
    by a DVE multiply with a 0/1 tri mask (was: 2 seed matmuls + 2 split S
    matmuls per diagonal block, with iden/negtri LDWEIGHTS churn)
  - softmax normalize fused: reciprocal_approx_fast (5x faster than the 6
    cyc/elem DVE reciprocal) on the denominator + scalar_tensor_tensor
    reading the attention half straight from PSUM (was: 2 evac copies +
    3.3us reciprocal + gpsimd multiply per half)
  - v stored per-h2-plane ([v|ones] for even heads, [ones|v] for odd) so the
    AV output lands attn on the same partitions the out-projection reads,
    letting the fused normalize write aT directly
  - out-projection PSUM evacuation on gpsimd; output stored bf16
"""

import os
from contextlib import ExitStack

import numpy as np
import ml_dtypes

B, T, C = 4, 2048, 1024
H, D = 16, 64
HG = 8            # heads per core
NCORES = 8
TB = T // 128     # 16 t/s-blocks of 128
CBN = C // 128    # 8 contraction chunks
NP = HG // 2      # 4 head pairs
NI = T // 512     # 4 t-windows of 512
ROPE_BASE = 10000.0

_PROG = None
_LAST_RESULTS = None


def _build_program():
    import concourse.bass as bass
    import concourse.tile as tile
    from concourse import bacc, mybir

    f32 = mybir.dt.float32
    bf = mybir.dt.bfloat16
    EXP = mybir.ActivationFunctionType.Exp
    MUL = mybir.AluOpType.mult

    nc = bacc.Bacc("TRN2", target_bir_lowering=False, debug=False)

    xT = nc.dram_tensor("xT", [C, T], bf, kind="ExternalInput").ap()
    wqkv = nc.dram_tensor("wqkv", [C, 3 * HG * D], bf, kind="ExternalInput").ap()
    wout = nc.dram_tensor("wout", [HG * D, C], bf, kind="ExternalInput").ap()
    cosc = nc.dram_tensor("cosc", [T, D], bf, kind="ExternalInput").ap()
    sinc = nc.dram_tensor("sinc", [T, D], bf, kind="ExternalInput").ap()
    padb = nc.dram_tensor("padb", [128, TB], f32, kind="ExternalInput").ap()
    tri = nc.dram_tensor("tri", [128, 128], bf, kind="ExternalInput").ap()
    outp = nc.dram_tensor("outp", [T, C], bf, kind="ExternalOutput").ap()

    with tile.TileContext(nc) as tc, ExitStack() as ctx:
        singles = ctx.enter_context(tc.tile_pool(name="singles", bufs=1))

        # ---- global SBUF tensors.  Loads are split across three DMA queues
        # (gpsimd: xT, vector: wqkv, scalar: the rest) and issued in the
        # order compute consumes them, so the first qkv matmul can start
        # ~2us in instead of waiting out one serial queue.  The Sync queue
        # carries ONLY DMA transposes so the xbar never flips modes. ----
        # Load-queue plan: single merged tiles so each load is ONE DMA
        # trigger (triggers cost ~0.7us gpsimd/sync, ~1.3us scalar, and
        # serialize per queue).  Segment 0's inputs all land by ~6us:
        #   sync:   w halves 0:4      (before the first transpose is issued,
        #                              so the xbar never flips modes)
        #   scalar: w halves 4:8, cos, sin, wout
        #   gpsimd: xt q0, padb, tri, xt q1..q3  (t-quarters in consumption
        #                              order: segment I only needs quarter I)
        xt_sb = []
        w_sb = []
        for cb in range(CBN):
            t_ = singles.tile([128, T], bf, name=f"xt{cb}", tag=f"xt{cb}")
            xt_sb.append(t_)
            t_ = singles.tile([128, 3 * HG * D], bf, name=f"w{cb}", tag=f"w{cb}")
            eng = nc.sync if cb < 6 else nc.scalar
            eng.dma_start(out=t_, in_=wqkv[cb * 128:(cb + 1) * 128, :])
            w_sb.append(t_)
        cos_sb = singles.tile([128, TB, D], bf, name="cos_sb", tag="cos_sb")
        nc.scalar.dma_start(
            out=cos_sb, in_=cosc.rearrange("(tb p) d -> p tb d", p=128))
        sin_sb = singles.tile([128, TB, D], bf, name="sin_sb", tag="sin_sb")
        nc.scalar.dma_start(
            out=sin_sb, in_=sinc.rearrange("(tb p) d -> p tb d", p=128))
        def xt_load_quarter(q):
            for cb in range(CBN):
                nc.gpsimd.dma_start(
                    out=xt_sb[cb][:, q * 512:(q + 1) * 512],
                    in_=xT[cb * 128:(cb + 1) * 128, q * 512:(q + 1) * 512])

        # quarters 0/1 up front; 2/3 are emitted at the heads of windows
        # 0/1 (one window before their segment drips) so their 8-trigger
        # bursts don't pile onto the gpsimd queue ahead of window 0's
        # latency-critical trimask ops
        xt_load_quarter(0)
        padb_sb = singles.tile([128, TB], f32, name="padb_sb", tag="padb_sb")
        nc.gpsimd.dma_start(out=padb_sb, in_=padb)
        tri_sb = singles.tile([128, 128], bf, name="tri_sb", tag="tri_sb")
        nc.gpsimd.dma_start(out=tri_sb, in_=tri)
        xt_load_quarter(1)
        wo_sb = []
        for c in range(4):
            t_ = singles.tile([128, C], bf, name=f"wo{c}", tag=f"wo{c}")
            nc.scalar.dma_start(out=t_, in_=wout[c * 128:(c + 1) * 128, :])
            wo_sb.append(t_)

        # q^T/k^T: [within-pair col (head-lo d / head-hi d), s-block, pair, t]
        qT_all = singles.tile([128, TB, NP, 128], bf, name="qT_all", tag="qT_all")
        kT_all = singles.tile([128, TB, NP, 128], bf, name="kT_all", tag="kT_all")
        # v packed per h2-plane: plane 0 (even heads) [v | ones], plane 1
        # (odd heads) [ones | v].  The ones columns make the AV matmul emit
        # the softmax denominator on the opposite 64 partitions from attn,
        # and attn lands exactly on the partitions aT/out-proj read, so the
        # fused normalize (recip + scalar_tensor_tensor) writes aT directly.
        vones = singles.tile([128, TB, 2, NP, 128], bf, name="vones", tag="vones")
        nc.vector.memset(vones[:, :, 0, :, D:128], 1.0)
        nc.vector.memset(vones[:, :, 1, :, 0:D], 1.0)

        # qkv + attention interleaved per 512-wide t-window so the PE stream
        # stays dense (HAM stays at 2.4 GHz): attention for window I only
        # needs q/k/v blocks 0..4I+3, which segment I of the qkv loop topped
        # off.  One shared PSUM pool: qkv 2 + sAB 2x2 + psT 2 = 8 banks.
        with tc.tile_pool(name="psum", bufs=2, space="PSUM") as psum, \
             tc.tile_pool(name="rope", bufs=4) as rope_pool, \
             tc.tile_pool(name="qknat", bufs=3) as qk_pool, \
             tc.tile_pool(name="exps", bufs=3) as exp_pool, \
             tc.tile_pool(name="attnT", bufs=4) as aT_pool, \
             tc.tile_pool(name="recips", bufs=2) as rc_pool, \
             tc.tile_pool(name="outsb", bufs=4) as out_pool:
            # segment 0 runs with no attention to hide behind, so PE<->DVE
            # round trips on a 2-deep PSUM ring would halve its duty cycle.
            # The attention banks (sAB/avA/avB) are idle until window 0 --
            # borrow them for a ~6-deep ring during segment 0 only.
            SEG0_TAGS = (("qkv", 2), ("sAB", 2), ("avA", 1), ("avB", 1))

            def qkv_segment(I):
                """Generator emitting segment I's qkv matmuls in half-tensor
                chunks (yield points), so the caller can drip them into the
                attention loop as PE gap-filler while ACT grinds exps.

                q units first, then k, then v: window I+1's FIRST S matmul's
                moving operand spans all 4 new qT blocks, so the q
                transposes must clear the sync queue as early as possible
                (k blocks j>=4I+4 and v aren't touched until ~20us into the
                window)."""
                nchunk = 0
                for which, base in (("q", 0), ("k", 512), ("v", 1024)):
                    for tb in range(4 * I, 4 * I + 4):
                        if I == 0:
                            tg, nb = SEG0_TAGS[nchunk % len(SEG0_TAGS)]
                            nchunk += 1
                            ps = psum.tile([128, HG, D], f32,
                                           name=f"ps{which}", tag=tg, bufs=nb)
                        else:
                            ps = psum.tile([128, HG, D], f32, name=f"ps{which}", tag="qkv")
                        for cb in range(CBN):
                            nc.tensor.matmul(
                                ps, xt_sb[cb][:, tb * 128:(tb + 1) * 128],
                                w_sb[cb][:, base:base + 512],
                                start=(cb == 0), stop=(cb == CBN - 1))
                            if cb == 3:
                                yield
                        if which == "v":
                            # v columns were host-reordered [evens | odds].
                            # Early segments evacuate on ACT (idle then);
                            # late ones on DVE (ACT is exp-bound there).
                            if I <= 1:
                                nc.scalar.copy(
                                    out=vones[:, tb, 0, :, 0:D], in_=ps[:, 0:NP, :])
                                nc.scalar.copy(
                                    out=vones[:, tb, 1, :, D:128], in_=ps[:, NP:HG, :])
                            else:
                                nc.vector.tensor_copy(
                                    out=vones[:, tb, 0, :, 0:D], in_=ps[:, 0:NP, :])
                                nc.vector.tensor_copy(
                                    out=vones[:, tb, 1, :, D:128], in_=ps[:, NP:HG, :])
                            yield
                            continue
                        # rope: P_c = qkv*cos, P_s = qkv*sin (compact [t,64]
                        # tables broadcast across the 8 heads), then
                        # lo = P_c.lo - P_s.hi ; hi = P_s.lo + P_c.hi
                        cosb = cos_sb[:, tb].unsqueeze(1).broadcast_to([128, HG, D])
                        sinb = sin_sb[:, tb].unsqueeze(1).broadcast_to([128, HG, D])
                        pc = rope_pool.tile([128, HG, D], f32, name="pc", tag="rt")
                        psn = rope_pool.tile([128, HG, D], f32, name="psn", tag="rt")
                        nc.vector.tensor_mul(pc, ps, cosb)
                        nc.vector.tensor_mul(psn, ps, sinb)
                        ro = qk_pool.tile([128, HG, D], bf, name="ro", tag="ro")
                        nc.vector.tensor_sub(
                            ro[:, :, 0:32], pc[:, :, 0:32], psn[:, :, 32:64])
                        nc.vector.tensor_add(
                            ro[:, :, 32:64], psn[:, :, 0:32], pc[:, :, 32:64])
                        dst = qT_all if which == "q" else kT_all
                        nc.sync.dma_start_transpose(out=dst[:, tb, :, :], in_=ro)
                        yield

            pending_out = []
            for I in range(NI):
                if I + 2 < NI:
                    xt_load_quarter(I + 2)
                nxt = qkv_segment(I + 1) if I + 1 < NI else None
                n_chunks = 24  # yield points per segment
                n_iters = 4 * (4 * I + 4)
                emitted = it = 0
                if I == 0:
                    for _ in qkv_segment(0):
                        pass
                    # window 0 is too short to absorb all of segment 1's
                    # matmul->rope->transpose pipeline; pre-drip half of it
                    # while the PE would otherwise idle waiting on window 0's
                    # first exps
                    while emitted < 12:
                        if next(nxt, "done") == "done":
                            break
                        emitted += 1

                def drip():
                    nonlocal emitted
                    got = 0
                    if nxt is None:
                        return got
                    # finish the segment by ~75% of the window (45% for the
                    # short window 0) so the next window's S matmuls never
                    # wait on the trailing rope->transpose pipeline
                    frac = 0.45 if I == 0 else 0.75
                    due = int(it * n_chunks / (frac * n_iters))
                    while emitted < due:
                        if next(nxt, "done") == "done":
                            break
                        emitted += 1
                        got += 1
                    return got

                # ---- attention window I ----
                aT_I = aT_pool.tile([128, NP, 512], bf, name="aT_I", tag="aT_I")
                for p in range(NP):
                    if nxt is None and p % 2 == 1:
                        # the last window has no qkv drip, so its 2 PSUM
                        # banks are free: odd pairs accumulate there, double-
                        # buffering the AV accumulators across pairs
                        psTA = psum.tile([128, 512], f32, name="psTA", tag="qkv", bufs=2)
                        psTB = psum.tile([128, 512], f32, name="psTB", tag="qkv", bufs=2)
                    else:
                        psTA = psum.tile([128, 512], f32, name="psTA", tag="avA", bufs=1)
                        psTB = psum.tile([128, 512], f32, name="psTB", tag="avB", bufs=1)

                    def emit_av(j, eAB):
                        off = max(j - 4 * I, 0) * 128
                        for h2, psT in ((0, psTA), (1, psTB)):
                            nc.tensor.matmul(
                                psT[:, off:512],
                                vones[:, j, h2, p, :],
                                eAB[:, h2, off:512],
                                start=(j == 0), stop=(j == 4 * I + 3))

                    prev = None
                    for j in range(4 * I + 4):
                        jl = j - 4 * I
                        off = max(jl, 0) * 128
                        sAB = psum.tile([128, 2, 512], f32, name="sAB", tag="sAB", bufs=2)
                        HALVES = ((0, slice(0, 64)), (1, slice(64, 128)))
                        # keep the row-tiled pair adjacent so the two heads
                        # overlap in the PE array; diagonal block computed
                        # unmasked (masked post-exp on DVE)
                        lo = max(jl, 0)
                        for h2, rows in HALVES:
                            nc.tensor.matmul(
                                sAB[:, h2, off:512],
                                kT_all[rows, j, p, :],
                                qT_all[rows, 4 * I + lo:4 * I + 4, p, :],
                                start=True, stop=True,
                                tile_position=(h2 * 64, 0))
                        eAB = exp_pool.tile([128, 2, 512], bf, name="eAB", tag="eAB")
                        bias = padb_sb[:, j:j + 1]
                        nc.scalar.activation(
                            out=eAB[:, :, off:512], in_=sAB[:, :, off:512],
                            func=EXP, bias=bias, scale=0.125)
                        if jl >= 0:
                            # zero exp(S) above the diagonal of the diagonal
                            # block (replaces the -1e30 seed matmuls); on
                            # gpsimd -- it is idle during windows and this
                            # keeps DVE off the S->AV critical path
                            trib = tri_sb.unsqueeze(1).broadcast_to([128, 2, 128])
                            nc.gpsimd.tensor_mul(
                                eAB[:, :, off:off + 128],
                                eAB[:, :, off:off + 128], trib)
                        if prev is not None:
                            emit_av(*prev)
                        prev = (j, eAB)
                        it += 1
                        # all out-projection units are deferred to the LAST
                        # window: exp work grows with causal depth, so window
                        # 3 is ACT-bound with ~24us of PE idle that the OP
                        # matmuls fill (earlier windows are PE-bound from the
                        # qkv drip).  Keep 2 units back for the tail so the
                        # PE has work while the final normalize chain drains.
                        if (nxt is None and it % 2 == 0
                                and len(pending_out) > 2):
                            pending_out.pop(0)()
                        drip()
                    emit_av(*prev)

                    # fused softmax normalize, straight out of PSUM into the
                    # out-projection's stationary layout: plane layout puts
                    # attn on the partitions aT reads (0:64 for h2=0, 64:128
                    # for h2=1) and the replicated denominator on the other
                    # 64.  Shift-copy the denominator onto attn's partitions,
                    # approx-reciprocal it (~51 ULP, plenty for softmax), then
                    # aT = psum_attn * recip in one scalar_tensor_tensor.
                    # (reciprocal_approx_fast silently breaks on partition-
                    # sliced APs, so gather both denominators into one tile
                    # and reciprocal all 128 partitions in one op)
                    cpd = rc_pool.tile([128, 512], f32, name="cpd", tag="rcd")
                    rc = rc_pool.tile([128, 512], f32, name="rc", tag="rc")
                    nc.vector.tensor_copy(out=cpd[0:64, :], in_=psTA[64:128, :])
                    nc.vector.tensor_copy(out=cpd[64:128, :], in_=psTB[0:64, :])
                    nc.vector.reciprocal_approx_fast(rc, cpd)
                    for h2, psT in ((0, psTA), (1, psTB)):
                        half = slice(h2 * 64, h2 * 64 + 64)
                        nc.vector.scalar_tensor_tensor(
                            out=aT_I[half, p, :], in0=psT[half, :], scalar=1.0,
                            in1=rc[half, :], op0=MUL, op1=MUL)

                # out-projection units are deferred into the next window's
                # loop as more PE gap-filler
                def make_out_unit(aT, i, il, n):
                    def emit():
                        pso = psum.tile([128, 512], f32, name="pso", tag="sAB", bufs=2)
                        for c in range(4):
                            nc.tensor.matmul(
                                pso,
                                aT[:, c, il * 128:(il + 1) * 128],
                                wo_sb[c][:, n * 512:(n + 1) * 512],
                                start=(c == 0), stop=(c == 3))
                        # evacuate on DVE: the units run in the ACT-bound
                        # last window, where ACT (exp) must not be loaded
                        # further and DVE has slack.  Stores alternate
                        # between the gpsimd and sync DMA queues (sync's
                        # transposes are all done by mid-window 2) so the
                        # deferred-store backlog doesn't stall the tail.
                        osb = out_pool.tile([128, 512], bf, name="osb", tag="osb")
                        nc.vector.tensor_copy(out=osb, in_=pso)
                        seng = nc.gpsimd if (i + n) % 2 == 0 else nc.sync
                        seng.dma_start(
                            out=outp[i * 128:(i + 1) * 128, n * 512:(n + 1) * 512],
                            in_=osb)
                    return emit
                for il in range(4):
                    for n in range(2):
                        pending_out.append(make_out_unit(aT_I, 4 * I + il, il, n))
                if nxt is not None:
                    for _ in nxt:
                        pass
            for f in pending_out:
                f()

    nc.compile()
    return nc


def _get_program():
    global _PROG
    if _PROG is None:
        _PROG = _build_program()
    return _PROG


def _rope_tables():
    bf16 = ml_dtypes.bfloat16
    inv = 1.0 / (ROPE_BASE ** (np.arange(0, D, 2, dtype=np.float64) / D))
    f = np.arange(T, dtype=np.float64)[:, None] * inv[None, :]  # [T, 32]
    c = np.cos(f)
    s = np.sin(f)
    # both 32-col halves carry the same table value
    cosc = np.concatenate([c, c], axis=1).astype(bf16)  # [T, 64]
    sinc = np.concatenate([s, s], axis=1).astype(bf16)
    return cosc, sinc


def kernel(x, attention_mask, W_qkv, W_out, b_out):
    global _LAST_RESULTS
    from concourse.bass_utils import run_bass_kernel_spmd

    nc = _get_program()
    bf16 = ml_dtypes.bfloat16
    x = np.asarray(x, dtype=np.float32)
    attention_mask = np.asarray(attention_mask)
    W_qkv = np.asarray(W_qkv, dtype=np.float32)
    W_out = np.asarray(W_out, dtype=np.float32)
    b_out = np.asarray(b_out, dtype=np.float32)

    cosc, sinc = _rope_tables()
    tri01 = np.where(np.arange(128)[:, None] > np.arange(128)[None, :], 0.0, 1.0)
    tri01 = tri01.astype(bf16)

    in_maps = []
    for g in range(NCORES):
        b, hg = g // 2, g % 2
        sl = slice(hg * 512, hg * 512 + 512)
        wq = W_qkv[:, 0 * C:][:, sl]
        wk = W_qkv[:, 1 * C:2 * C][:, sl]
        wv = W_qkv[:, 2 * C:3 * C][:, sl]
        # v columns reordered [even heads | odd heads] so the on-chip v
        # copies are two contiguous strided moves into the h2 planes
        wv = wv.reshape(C, HG, D)[:, [0, 2, 4, 6, 1, 3, 5, 7], :].reshape(C, 512)
        wqkv_g = np.ascontiguousarray(
            np.concatenate([wq, wk, wv], axis=1)).astype(bf16)
        xT_g = np.ascontiguousarray(x[b].T).astype(bf16)
        wout_g = np.ascontiguousarray(W_out[sl, :]).astype(bf16)
        padb_g = np.ascontiguousarray(
            np.where(attention_mask[b] != 0, 0.0, -1e30)
            .astype(np.float32).reshape(TB, 128).T)
        in_maps.append({
            "xT": xT_g, "wqkv": wqkv_g, "wout": wout_g,
            "cosc": cosc, "sinc": sinc, "padb": padb_g, "tri": tri01,
        })

    res = run_bass_kernel_spmd(nc, in_maps, list(range(NCORES)))
    _LAST_RESULTS = res
    out = np.empty((B, T, C), dtype=np.float32)
    for b in range(B):
        out[b] = (res.results[2 * b]["outp"].astype(np.float32)
                  + res.results[2 * b + 1]["outp"].astype(np.float32) + b_out)
    return out


# revision 31
# speedup vs baseline: 1.0544x; 1.0499x over previous
"""Causal self-attention (RoPE, 16 heads, d=64, B=4, T=2048, C=1024) on 8 TRN2 cores.

Sharding: core g = (batch b = g//2, head-group hg = g%2 covering 8 heads).
Data-parallel over B, tensor-parallel over heads.  Each core computes the
partial out-projection (its 8 heads' contribution, no bias); the host sums
the two head-group partials per batch and adds b_out.

v1 changes over the 419.5us baseline:
  - input loads split across gpsimd/vector/scalar DMA queues in need-order
    (was: one serial gpsimd queue -> ~40us dead start)
  - cos/sin tables stored compact [T, 64] and broadcast across heads in the
    rope multiplies (was: 8x duplicated, 4MB of DMA)
  - diagonal-block masking moved post-exp: S computed unmasked in one merged
    matmul pair per diagonal j, then the upper-triangle of exp(S) is zeroed
    by a DVE multiply with a 0/1 tri mask (was: 2 seed matmuls + 2 split S
    matmuls per diagonal block, with iden/negtri LDWEIGHTS churn)
  - softmax normalize fused: reciprocal_approx_fast (5x faster than the 6
    cyc/elem DVE reciprocal) on the denominator + scalar_tensor_tensor
    reading the attention half straight from PSUM (was: 2 evac copies +
    3.3us reciprocal + gpsimd multiply per half)
  - v stored per-h2-plane ([v|ones] for even heads, [ones|v] for odd) so the
    AV output lands attn on the same partitions the out-projection reads,
    letting the fused normalize write aT directly
  - out-projection PSUM evacuation on gpsimd; output stored bf16
"""

import os
from contextlib import ExitStack

import numpy as np
import ml_dtypes

B, T, C = 4, 2048, 1024
H, D = 16, 64
HG = 8            # heads per core
NCORES = 8
TB = T // 128     # 16 t/s-blocks of 128
CBN = C // 128    # 8 contraction chunks
NP = HG // 2      # 4 head pairs
NI = T // 512     # 4 t-windows of 512
ROPE_BASE = 10000.0

_PROG = None
_LAST_RESULTS = None


def _build_program():
    import concourse.bass as bass
    import concourse.tile as tile
    from concourse import bacc, mybir

    f32 = mybir.dt.float32
    bf = mybir.dt.bfloat16
    EXP = mybir.ActivationFunctionType.Exp
    MUL = mybir.AluOpType.mult

    nc = bacc.Bacc("TRN2", target_bir_lowering=False, debug=False)

    xT = nc.dram_tensor("xT", [C, T], bf, kind="ExternalInput").ap()
    wqkv = nc.dram_tensor("wqkv", [C, 3 * HG * D], bf, kind="ExternalInput").ap()
    wout = nc.dram_tensor("wout", [HG * D, C], bf, kind="ExternalInput").ap()
    cosc = nc.dram_tensor("cosc", [T, D], bf, kind="ExternalInput").ap()
    sinc = nc.dram_tensor("sinc", [T, D], bf, kind="ExternalInput").ap()
    padb = nc.dram_tensor("padb", [128, TB], f32, kind="ExternalInput").ap()
    tri = nc.dram_tensor("tri", [128, 128], bf, kind="ExternalInput").ap()
    outp = nc.dram_tensor("outp", [T, C], bf, kind="ExternalOutput").ap()

    with tile.TileContext(nc) as tc, ExitStack() as ctx:
        singles = ctx.enter_context(tc.tile_pool(name="singles", bufs=1))

        # ---- global SBUF tensors.  Loads are split across three DMA queues
        # (gpsimd: xT, vector: wqkv, scalar: the rest) and issued in the
        # order compute consumes them, so the first qkv matmul can start
        # ~2us in instead of waiting out one serial queue.  The Sync queue
        # carries ONLY DMA transposes so the xbar never flips modes. ----
        # Load-queue plan: single merged tiles so each load is ONE DMA
        # trigger (triggers cost ~0.7us gpsimd/sync, ~1.3us scalar, and
        # serialize per queue).  Segment 0's inputs all land by ~6us:
        #   sync:   w halves 0:4      (before the first transpose is issued,
        #                              so the xbar never flips modes)
        #   scalar: w halves 4:8, cos, sin, wout
        #   gpsimd: xt q0, padb, tri, xt q1..q3  (t-quarters in consumption
        #                              order: segment I only needs quarter I)
        # DMA completion-semaphores come from a small shared pool; every
        # extra DMA forces a later DMA (incl. the latency-critical
        # transposes) to wait for its slot's previous owner.  So: per-chunk
        # DMAs ONLY where the startup needs fine-grained completion (w, xt
        # quarter 0), single merged DMAs for everything consumed later.
        w_sb = []
        for cb in range(CBN):
            t_ = singles.tile([128, 3 * HG * D], bf, name=f"w{cb}", tag=f"w{cb}")
            eng = nc.sync if cb < 6 else nc.scalar
            eng.dma_start(out=t_, in_=wqkv[cb * 128:(cb + 1) * 128, :])
            w_sb.append(t_)
        cos_sb = singles.tile([128, TB, D], bf, name="cos_sb", tag="cos_sb")
        nc.scalar.dma_start(
            out=cos_sb, in_=cosc.rearrange("(tb p) d -> p tb d", p=128))
        sin_sb = singles.tile([128, TB, D], bf, name="sin_sb", tag="sin_sb")
        nc.scalar.dma_start(
            out=sin_sb, in_=sinc.rearrange("(tb p) d -> p tb d", p=128))
        xt_all = singles.tile([128, CBN, T], bf, name="xt_all", tag="xt_all")
        for cb in range(CBN):
            nc.gpsimd.dma_start(
                out=xt_all[:, cb, 0:512],
                in_=xT[cb * 128:(cb + 1) * 128, 0:512])
        padb_sb = singles.tile([128, TB], f32, name="padb_sb", tag="padb_sb")
        nc.gpsimd.dma_start(out=padb_sb, in_=padb)
        tri_sb = singles.tile([128, 128], bf, name="tri_sb", tag="tri_sb")
        nc.gpsimd.dma_start(out=tri_sb, in_=tri)
        xx = xT.rearrange("(cb p) t -> p cb t", p=128)
        for q in range(1, NI):
            nc.gpsimd.dma_start(
                out=xt_all[:, :, q * 512:(q + 1) * 512],
                in_=xx[:, :, q * 512:(q + 1) * 512])
        xt_sb = [xt_all[:, cb] for cb in range(CBN)]
        wo_all = singles.tile([128, 4, C], bf, name="wo_all", tag="wo_all")
        nc.scalar.dma_start(
            out=wo_all, in_=wout.rearrange("(c p) n -> p c n", p=128))
        wo_sb = [wo_all[:, c] for c in range(4)]

        # q^T/k^T: [within-pair col (head-lo d / head-hi d), s-block, pair, t]
        qT_all = singles.tile([128, TB, NP, 128], bf, name="qT_all", tag="qT_all")
        kT_all = singles.tile([128, TB, NP, 128], bf, name="kT_all", tag="kT_all")
        # v packed per h2-plane: plane 0 (even heads) [v | ones], plane 1
        # (odd heads) [ones | v].  The ones columns make the AV matmul emit
        # the softmax denominator on the opposite 64 partitions from attn,
        # and attn lands exactly on the partitions aT/out-proj read, so the
        # fused normalize (recip + scalar_tensor_tensor) writes aT directly.
        vones = singles.tile([128, TB, 2, NP, 128], bf, name="vones", tag="vones")
        nc.vector.memset(vones[:, :, 0, :, D:128], 1.0)
        nc.vector.memset(vones[:, :, 1, :, 0:D], 1.0)

        # qkv + attention interleaved per 512-wide t-window so the PE stream
        # stays dense (HAM stays at 2.4 GHz): attention for window I only
        # needs q/k/v blocks 0..4I+3, which segment I of the qkv loop topped
        # off.  One shared PSUM pool: qkv 2 + sAB 2x2 + psT 2 = 8 banks.
        with tc.tile_pool(name="psum", bufs=2, space="PSUM") as psum, \
             tc.tile_pool(name="rope", bufs=4) as rope_pool, \
             tc.tile_pool(name="qknat", bufs=3) as qk_pool, \
             tc.tile_pool(name="exps", bufs=3) as exp_pool, \
             tc.tile_pool(name="attnT", bufs=4) as aT_pool, \
             tc.tile_pool(name="recips", bufs=2) as rc_pool, \
             tc.tile_pool(name="outsb", bufs=4) as out_pool:
            # segment 0 runs with no attention to hide behind, so PE<->DVE
            # round trips on a 2-deep PSUM ring would halve its duty cycle.
            # The attention banks (sAB/avA/avB) are idle until window 0 --
            # borrow them for a ~6-deep ring during segment 0 only.
            SEG0_TAGS = (("qkv", 2), ("sAB", 2), ("avA", 1), ("avB", 1))

            def qkv_segment(I):
                """Generator emitting segment I's qkv matmuls in half-tensor
                chunks (yield points), so the caller can drip them into the
                attention loop as PE gap-filler while ACT grinds exps.

                q units first, then k, then v: window I+1's FIRST S matmul's
                moving operand spans all 4 new qT blocks, so the q
                transposes must clear the sync queue as early as possible
                (k blocks j>=4I+4 and v aren't touched until ~20us into the
                window)."""
                nchunk = 0
                for which, base in (("q", 0), ("k", 512), ("v", 1024)):
                    for tb in range(4 * I, 4 * I + 4):
                        if I == 0:
                            tg, nb = SEG0_TAGS[nchunk % len(SEG0_TAGS)]
                            nchunk += 1
                            ps = psum.tile([128, HG, D], f32,
                                           name=f"ps{which}", tag=tg, bufs=nb)
                        else:
                            ps = psum.tile([128, HG, D], f32, name=f"ps{which}", tag="qkv")
                        for cb in range(CBN):
                            nc.tensor.matmul(
                                ps, xt_sb[cb][:, tb * 128:(tb + 1) * 128],
                                w_sb[cb][:, base:base + 512],
                                start=(cb == 0), stop=(cb == CBN - 1))
                            if cb == 3:
                                yield
                        if which == "v":
                            # v columns were host-reordered [evens | odds].
                            # Early segments evacuate on ACT (idle then);
                            # late ones on DVE (ACT is exp-bound there).
                            if I <= 1:
                                nc.scalar.copy(
                                    out=vones[:, tb, 0, :, 0:D], in_=ps[:, 0:NP, :])
                                nc.scalar.copy(
                                    out=vones[:, tb, 1, :, D:128], in_=ps[:, NP:HG, :])
                            else:
                                nc.vector.tensor_copy(
                                    out=vones[:, tb, 0, :, 0:D], in_=ps[:, 0:NP, :])
                                nc.vector.tensor_copy(
                                    out=vones[:, tb, 1, :, D:128], in_=ps[:, NP:HG, :])
                            yield
                            continue
                        # rope: P_c = qkv*cos, P_s = qkv*sin (compact [t,64]
                        # tables broadcast across the 8 heads), then
                        # lo = P_c.lo - P_s.hi ; hi = P_s.lo + P_c.hi
                        cosb = cos_sb[:, tb].unsqueeze(1).broadcast_to([128, HG, D])
                        sinb = sin_sb[:, tb].unsqueeze(1).broadcast_to([128, HG, D])
                        pc = rope_pool.tile([128, HG, D], f32, name="pc", tag="rt")
                        psn = rope_pool.tile([128, HG, D], f32, name="psn", tag="rt")
                        nc.vector.tensor_mul(pc, ps, cosb)
                        nc.vector.tensor_mul(psn, ps, sinb)
                        ro = qk_pool.tile([128, HG, D], bf, name="ro", tag="ro")
                        nc.vector.tensor_sub(
                            ro[:, :, 0:32], pc[:, :, 0:32], psn[:, :, 32:64])
                        nc.vector.tensor_add(
                            ro[:, :, 32:64], psn[:, :, 0:32], pc[:, :, 32:64])
                        dst = qT_all if which == "q" else kT_all
                        nc.sync.dma_start_transpose(out=dst[:, tb, :, :], in_=ro)
                        yield

            pending_out = []
            for I in range(NI):
                nxt = qkv_segment(I + 1) if I + 1 < NI else None
                n_chunks = 24  # yield points per segment
                n_iters = 4 * (4 * I + 4)
                emitted = it = 0
                if I == 0:
                    for _ in qkv_segment(0):
                        pass
                    # window 0 is too short to absorb all of segment 1's
                    # matmul->rope->transpose pipeline; pre-drip half of it
                    # while the PE would otherwise idle waiting on window 0's
                    # first exps
                    while emitted < 12:
                        if next(nxt, "done") == "done":
                            break
                        emitted += 1

                def drip():
                    nonlocal emitted
                    got = 0
                    if nxt is None:
                        return got
                    # finish the segment by ~75% of the window (45% for the
                    # short window 0) so the next window's S matmuls never
                    # wait on the trailing rope->transpose pipeline
                    frac = 0.45 if I == 0 else 0.75
                    due = int(it * n_chunks / (frac * n_iters))
                    while emitted < due:
                        if next(nxt, "done") == "done":
                            break
                        emitted += 1
                        got += 1
                    return got

                # ---- attention window I ----
                aT_I = aT_pool.tile([128, NP, 512], bf, name="aT_I", tag="aT_I")
                for p in range(NP):
                    if nxt is None and p % 2 == 1:
                        # the last window has no qkv drip, so its 2 PSUM
                        # banks are free: odd pairs accumulate there, double-
                        # buffering the AV accumulators across pairs
                        psTA = psum.tile([128, 512], f32, name="psTA", tag="qkv", bufs=2)
                        psTB = psum.tile([128, 512], f32, name="psTB", tag="qkv", bufs=2)
                    else:
                        psTA = psum.tile([128, 512], f32, name="psTA", tag="avA", bufs=1)
                        psTB = psum.tile([128, 512], f32, name="psTB", tag="avB", bufs=1)

                    def emit_av(j, eAB):
                        off = max(j - 4 * I, 0) * 128
                        for h2, psT in ((0, psTA), (1, psTB)):
                            nc.tensor.matmul(
                                psT[:, off:512],
                                vones[:, j, h2, p, :],
                                eAB[:, h2, off:512],
                                start=(j == 0), stop=(j == 4 * I + 3))

                    prev = None
                    for j in range(4 * I + 4):
                        jl = j - 4 * I
                        off = max(jl, 0) * 128
                        sAB = psum.tile([128, 2, 512], f32, name="sAB", tag="sAB", bufs=2)
                        HALVES = ((0, slice(0, 64)), (1, slice(64, 128)))
                        # keep the row-tiled pair adjacent so the two heads
                        # overlap in the PE array; diagonal block computed
                        # unmasked (masked post-exp on DVE)
                        lo = max(jl, 0)
                        for h2, rows in HALVES:
                            nc.tensor.matmul(
                                sAB[:, h2, off:512],
                                kT_all[rows, j, p, :],
                                qT_all[rows, 4 * I + lo:4 * I + 4, p, :],
                                start=True, stop=True,
                                tile_position=(h2 * 64, 0))
                        eAB = exp_pool.tile([128, 2, 512], bf, name="eAB", tag="eAB")
                        bias = padb_sb[:, j:j + 1]
                        nc.scalar.activation(
                            out=eAB[:, :, off:512], in_=sAB[:, :, off:512],
                            func=EXP, bias=bias, scale=0.125)
                        if jl >= 0:
                            # zero exp(S) above the diagonal of the diagonal
                            # block (replaces the -1e30 seed matmuls); on
                            # gpsimd -- it is idle during windows and this
                            # keeps DVE off the S->AV critical path
                            trib = tri_sb.unsqueeze(1).broadcast_to([128, 2, 128])
                            nc.gpsimd.tensor_mul(
                                eAB[:, :, off:off + 128],
                                eAB[:, :, off:off + 128], trib)
                        if prev is not None:
                            emit_av(*prev)
                        prev = (j, eAB)
                        it += 1
                        # all out-projection units are deferred to the LAST
                        # window: exp work grows with causal depth, so window
                        # 3 is ACT-bound with ~24us of PE idle that the OP
                        # matmuls fill (earlier windows are PE-bound from the
                        # qkv drip).  Keep 2 units back for the tail so the
                        # PE has work while the final normalize chain drains.
                        if (nxt is None and it % 2 == 0
                                and len(pending_out) > 2):
                            pending_out.pop(0)()
                        drip()
                    emit_av(*prev)

                    # fused softmax normalize, straight out of PSUM into the
                    # out-projection's stationary layout: plane layout puts
                    # attn on the partitions aT reads (0:64 for h2=0, 64:128
                    # for h2=1) and the replicated denominator on the other
                    # 64.  Shift-copy the denominator onto attn's partitions,
                    # approx-reciprocal it (~51 ULP, plenty for softmax), then
                    # aT = psum_attn * recip in one scalar_tensor_tensor.
                    # (reciprocal_approx_fast silently breaks on partition-
                    # sliced APs, so gather both denominators into one tile
                    # and reciprocal all 128 partitions in one op)
                    cpd = rc_pool.tile([128, 512], f32, name="cpd", tag="rcd")
                    rc = rc_pool.tile([128, 512], f32, name="rc", tag="rc")
                    nc.vector.tensor_copy(out=cpd[0:64, :], in_=psTA[64:128, :])
                    nc.vector.tensor_copy(out=cpd[64:128, :], in_=psTB[0:64, :])
                    nc.vector.reciprocal_approx_fast(rc, cpd)
                    for h2, psT in ((0, psTA), (1, psTB)):
                        half = slice(h2 * 64, h2 * 64 + 64)
                        nc.vector.scalar_tensor_tensor(
                            out=aT_I[half, p, :], in0=psT[half, :], scalar=1.0,
                            in1=rc[half, :], op0=MUL, op1=MUL)

                # out-projection units are deferred into the next window's
                # loop as more PE gap-filler
                def make_out_unit(aT, i, il, n):
                    def emit():
                        pso = psum.tile([128, 512], f32, name="pso", tag="sAB", bufs=2)
                        for c in range(4):
                            nc.tensor.matmul(
                                pso,
                                aT[:, c, il * 128:(il + 1) * 128],
                                wo_sb[c][:, n * 512:(n + 1) * 512],
                                start=(c == 0), stop=(c == 3))
                        # evacuate on DVE: the units run in the ACT-bound
                        # last window, where ACT (exp) must not be loaded
                        # further and DVE has slack.  Stores alternate
                        # between the gpsimd and sync DMA queues (sync's
                        # transposes are all done by mid-window 2) so the
                        # deferred-store backlog doesn't stall the tail.
                        osb = out_pool.tile([128, 512], bf, name="osb", tag="osb")
                        nc.vector.tensor_copy(out=osb, in_=pso)
                        seng = nc.gpsimd if (i + n) % 2 == 0 else nc.sync
                        seng.dma_start(
                            out=outp[i * 128:(i + 1) * 128, n * 512:(n + 1) * 512],
                            in_=osb)
                    return emit
                for il in range(4):
                    for n in range(2):
                        pending_out.append(make_out_unit(aT_I, 4 * I + il, il, n))
                if nxt is not None:
                    for _ in nxt:
                        pass
            for f in pending_out:
                f()

    nc.compile()
    return nc


def _get_program():
    global _PROG
    if _PROG is None:
        _PROG = _build_program()
    return _PROG


def _rope_tables():
    bf16 = ml_dtypes.bfloat16
    inv = 1.0 / (ROPE_BASE ** (np.arange(0, D, 2, dtype=np.float64) / D))
    f = np.arange(T, dtype=np.float64)[:, None] * inv[None, :]  # [T, 32]
    c = np.cos(f)
    s = np.sin(f)
    # both 32-col halves carry the same table value
    cosc = np.concatenate([c, c], axis=1).astype(bf16)  # [T, 64]
    sinc = np.concatenate([s, s], axis=1).astype(bf16)
    return cosc, sinc


def kernel(x, attention_mask, W_qkv, W_out, b_out):
    global _LAST_RESULTS
    from concourse.bass_utils import run_bass_kernel_spmd

    nc = _get_program()
    bf16 = ml_dtypes.bfloat16
    x = np.asarray(x, dtype=np.float32)
    attention_mask = np.asarray(attention_mask)
    W_qkv = np.asarray(W_qkv, dtype=np.float32)
    W_out = np.asarray(W_out, dtype=np.float32)
    b_out = np.asarray(b_out, dtype=np.float32)

    cosc, sinc = _rope_tables()
    tri01 = np.where(np.arange(128)[:, None] > np.arange(128)[None, :], 0.0, 1.0)
    tri01 = tri01.astype(bf16)

    in_maps = []
    for g in range(NCORES):
        b, hg = g // 2, g % 2
        sl = slice(hg * 512, hg * 512 + 512)
        wq = W_qkv[:, 0 * C:][:, sl]
        wk = W_qkv[:, 1 * C:2 * C][:, sl]
        wv = W_qkv[:, 2 * C:3 * C][:, sl]
        # v columns reordered [even heads | odd heads] so the on-chip v
        # copies are two contiguous strided moves into the h2 planes
        wv = wv.reshape(C, HG, D)[:, [0, 2, 4, 6, 1, 3, 5, 7], :].reshape(C, 512)
        wqkv_g = np.ascontiguousarray(
            np.concatenate([wq, wk, wv], axis=1)).astype(bf16)
        xT_g = np.ascontiguousarray(x[b].T).astype(bf16)
        wout_g = np.ascontiguousarray(W_out[sl, :]).astype(bf16)
        padb_g = np.ascontiguousarray(
            np.where(attention_mask[b] != 0, 0.0, -1e30)
            .astype(np.float32).reshape(TB, 128).T)
        in_maps.append({
            "xT": xT_g, "wqkv": wqkv_g, "wout": wout_g,
            "cosc": cosc, "sinc": sinc, "padb": padb_g, "tri": tri01,
        })

    res = run_bass_kernel_spmd(nc, in_maps, list(range(NCORES)))
    _LAST_RESULTS = res
    out = np.empty((B, T, C), dtype=np.float32)
    for b in range(B):
        out[b] = (res.results[2 * b]["outp"].astype(np.float32)
                  + res.results[2 * b + 1]["outp"].astype(np.float32) + b_out)
    return out
